# revision 1
# baseline (speedup 1.0000x reference)
"""AGCRN cell kernel for 8 Trainium2 NeuronCores.

Strategy: data-parallel over batch (B=32 -> 4 per core, no collectives).
Each core redundantly builds S = exp(relu(E E^T)) (symmetric, serving
directly as the chain matmul's stationary operand). The adaptive-adjacency
row sums come for free from a ones-column prepended to the chain rhs
(col 0 of every batch slot is 1.0), so the S-build is a pure
matmul -> ACT-exp(PSUM->SBUF) -> DVE-max(.,1) pipeline with no slow
PSUM-f32 DVE reads. Normalization 1/d folds into PSUM evacuations as a
per-partition scale. Logits are split-bf16 (K=30) for f32 precision.
The Chebyshev chain runs node-major with 268-wide moving operands
(4 batches x [1|x|state]); x_g lives in one 256-col-per-batch slot so the
x_g @ W contraction needs just two [128,128] chunk transposes per
(tile, batch) - done by the XBAR DMA-transpose engines (not the PE), with
the W chunks zero-padded to absorb slot padding and the ones column
providing the bias. Epilogue reads state from the f32 staging tile
directly (no bf16 state copy).
"""

import os
import sys

import numpy as np
import ml_dtypes

for _p in ("/opt/trn_rl_repo", "/root/.axon_site/_ro/trn_rl_repo"):
    if os.path.isdir(_p) and _p not in sys.path:
        sys.path.append(_p)

import concourse.bass as bass
import concourse.tile as tile
from concourse import bacc, mybir
from concourse.bass_utils import run_bass_kernel_spmd

F32 = mybir.dt.float32
BF16 = mybir.dt.bfloat16
AF = mybir.ActivationFunctionType
ALU = mybir.AluOpType

P = 128          # partitions
N = 2048         # nodes
NT = N // P      # node tiles = 16
NB = 4           # batches per core
CH = 66          # dim_in + hidden
CPB = 256        # per-batch slot: [1 | x(2) | state(64) | u1(66) | u2(66) | 0-pad]
HID = 64
OC_G = 128       # gate output channels (2*hidden)
NCORES = 8
# slot column offsets
C_ONE = 0
C_X0 = 1          # x at 1:3, state at 3:67
C_U1 = 67
C_U2 = 133
C_PAD = 199       # zeros 199:256


def build_nc():
    nc = bacc.Bacc(
        "TRN2",
        target_bir_lowering=False,
        debug=False,
        enable_asserts=False,
        num_devices=NCORES,
    )
    x_d = nc.dram_tensor("x", [NB, N, 2], F32, kind="ExternalInput").ap()
    st_d = nc.dram_tensor("state", [NB, N, HID], F32, kind="ExternalInput").ap()
    # [E_hi; E_lo; E_hi] vs [E_hi; E_hi; E_lo]: Eh.Eh + El.Eh + Eh.El ~ E.E
    et_d = nc.dram_tensor("et", [2, 30, N], BF16, kind="ExternalInput").ap()
    id_d = nc.dram_tensor("ident", [P, P], BF16, kind="ExternalInput").ap()
    wg_d = nc.dram_tensor("wg", [2, P, OC_G], BF16, kind="ExternalInput").ap()
    wu_d = nc.dram_tensor("wu", [2, P, HID], BF16, kind="ExternalInput").ap()
    out_d = nc.dram_tensor("out", [NB, N, HID], F32, kind="ExternalOutput").ap()

    with tile.TileContext(nc) as tc:
        _build(tc, x_d, st_d, et_d, id_d, wg_d, wu_d, out_d)
    nc.compile()
    return nc


def _build(tc, x_d, st_d, et_d, id_d, wg_d, wu_d, out_d):
    nc = tc.nc
    from contextlib import ExitStack

    with ExitStack() as ctx:
        const = ctx.enter_context(tc.tile_pool(name="const", bufs=1))
        persist = ctx.enter_context(tc.tile_pool(name="persist", bufs=1))

        # ~1.8us of dummy matmuls: pushes the PE HAM clock-gate to 8/8
        # before the S-build matmuls arrive.
        wzero = const.tile([P, P], BF16)
        nc.vector.memset(wzero[:], 0.0)
        with tc.tile_pool(name="warm", bufs=1, space="PSUM") as warm:
            wp = warm.tile([P, P], F32)
            for _ in range(18):
                nc.tensor.matmul(wp[:], lhsT=wzero[:], rhs=wzero[:], start=True, stop=True)

        ident = const.tile([P, P], BF16)
        nc.sync.dma_start(ident[:], id_d[:])
        etp = const.tile([30, 2, N], BF16)
        nc.scalar.dma_start(etp[:, 0, :], et_d[0])
        nc.scalar.dma_start(etp[:, 1, :], et_d[1])
        wg_sb = const.tile([P, 2, OC_G], BF16)
        wu_sb = const.tile([P, 2, HID], BF16)
        for k in range(2):
            nc.scalar.dma_start(wg_sb[:, k, :], wg_d[k])
            nc.scalar.dma_start(wu_sb[:, k, :], wu_d[k])

        S_sb = persist.tile([P, NT, N], BF16)            # Shat row-tiles
        xg_sb = persist.tile([P, NT, NB, CPB], BF16)     # [1|x|state|u1|u2|pad]
        stf = persist.tile([P, NT, NB, HID], F32)        # state f32 (epilogue too)
        xf = persist.tile([P, NT, NB, 2], F32)
        zr_sb = persist.tile([P, NT, NB, OC_G], BF16)    # sigmoid(gate)
        rinv = persist.tile([P, NT], F32)
        rinv2 = persist.tile([P, NT], F32)

        # ---- input loads (f32 staging) ----
        for b in range(NB):
            nc.sync.dma_start(stf[:, :, b, :], st_d[b].rearrange("(t p) h -> p t h", p=P))
            nc.sync.dma_start(xf[:, :, b, :], x_d[b].rearrange("(t p) c -> p t c", p=P))

        # ---- slot init: ones col, pad cols, x/state conversions ----
        nc.gpsimd.memset(xg_sb[:, :, :, C_ONE : C_ONE + 1], 1.0)
        nc.gpsimd.memset(xg_sb[:, :, :, C_PAD:CPB], 0.0)
        nc.vector.tensor_copy(xg_sb[:, :, :, C_X0 : C_X0 + 2], xf[:])
        for b in range(NB):
            if b % 2 == 0:
                nc.vector.tensor_copy(xg_sb[:, :, b, 3:67], stf[:, :, b, :])
            else:
                nc.scalar.activation(xg_sb[:, :, b, 3:67], stf[:, :, b, :], AF.Copy)

        # ---- Shat = max(exp(E E^T), 1) == exp(relu(E E^T)) ----
        with tc.tile_pool(name="lpsum", bufs=2, space="PSUM") as lpsum:
            for mt in range(NT):
                lp = lpsum.tile([P, N], F32)
                for q in range(4):
                    nc.tensor.matmul(
                        lp[:, q * 512 : (q + 1) * 512],
                        lhsT=etp[:, 0, mt * P : (mt + 1) * P],
                        rhs=etp[:, 1, q * 512 : (q + 1) * 512],
                        start=True,
                        stop=True,
                    )
                nc.scalar.activation(S_sb[:, mt, :], lp[:], AF.Exp)
                nc.vector.tensor_scalar_max(S_sb[:, mt, :], S_sb[:, mt, :], 1.0)

        cpsum = ctx.enter_context(tc.tile_pool(name="cpsum", bufs=3, space="PSUM"))
        zpsum = ctx.enter_context(tc.tile_pool(name="zpsum", bufs=2, space="PSUM"))
        tpsum = ctx.enter_context(tc.tile_pool(name="tpsum", bufs=3, space="PSUM"))
        xgt_pool = ctx.enter_context(tc.tile_pool(name="xgt", bufs=12))
        epi_pool = ctx.enter_context(tc.tile_pool(name="epi", bufs=6))

        def apply1(first):
            """u1 = (Shat @ [1|x0]) / d for all mt; col 0 gives d row sums."""
            for mt in range(NT):
                cp = cpsum.tile([P, NB, 67], F32, tag="cp", name=f"a1_{first}_{mt}")
                for kt in range(NT):
                    nc.tensor.matmul(
                        cp[:],
                        lhsT=S_sb[:, kt, mt * P : (mt + 1) * P],
                        rhs=xg_sb[:, kt, :, 0:67],
                        start=(kt == 0),
                        stop=(kt == NT - 1),
                    )
                if first:
                    nc.vector.reciprocal(rinv[:, mt : mt + 1], cp[:, 0, 0:1])
                nc.scalar.activation(
                    xg_sb[:, mt, :, C_U1 : C_U1 + CH],
                    cp[:, :, 1:67],
                    AF.Copy,
                    scale=rinv[:, mt : mt + 1],
                )

        def apply2(mt):
            """u2 = 2*(Shat @ u1)/d - x0 for one mt."""
            cp = cpsum.tile([P, NB, CH], F32, tag="cp", name=f"a2_{mt}")
            for kt in range(NT):
                nc.tensor.matmul(
                    cp[:],
                    lhsT=S_sb[:, kt, mt * P : (mt + 1) * P],
                    rhs=xg_sb[:, kt, :, C_U1 : C_U1 + CH],
                    start=(kt == 0),
                    stop=(kt == NT - 1),
                )
            nc.vector.scalar_tensor_tensor(
                out=xg_sb[:, mt, :, C_U2 : C_U2 + CH],
                in0=cp[:],
                scalar=rinv2[:, mt : mt + 1],
                in1=xg_sb[:, mt, :, C_X0 : C_X0 + CH],
                op0=ALU.mult,
                op1=ALU.subtract,
            )

        def tail_nt(nt, gate):
            """XBAR transposes + W matmul + nonlinearity (+ epilogue)."""
            w_sb = wg_sb if gate else wu_sb
            oc = OC_G if gate else HID
            xgts = []
            for b in range(NB):
                tp = tpsum.tile([P, 2, P], BF16, tag="tp", name=f"tp{nt}{b}")
                nc.tensor.transpose(tp[:, 0, :], xg_sb[:, nt, b, 0:128], ident[:])
                nc.tensor.transpose(tp[:, 1, :], xg_sb[:, nt, b, 128:256], ident[:])
                xgt = xgt_pool.tile([P, 2, P], BF16, tag="xgt", name=f"xg{nt}{b}")
                if b % 2 == 0:
                    nc.vector.tensor_copy(xgt[:], tp[:])
                else:
                    nc.scalar.activation(xgt[:], tp[:], AF.Copy)
                xgts.append(xgt)
            zp = zpsum.tile([P, NB, oc], F32, tag="zp", name=f"zp{nt}")
            for b in range(NB):
                for k in range(2):
                    nc.tensor.matmul(
                        zp[:, b, :],
                        lhsT=xgts[b][:, k, :],
                        rhs=w_sb[:, k, :],
                        start=(k == 0),
                        stop=(k == 1),
                    )
            if gate:
                nc.scalar.activation(zr_sb[:, nt], zp[:], AF.Sigmoid)
                # candidate: state cols *= z (in place, all b)
                x0c = xg_sb[:, nt, :, 3:67]
                nc.vector.tensor_mul(x0c, x0c, zr_sb[:, nt, :, 0:HID])
            else:
                hc = epi_pool.tile([P, NB, HID], BF16, tag="hc", name=f"hc{nt}")
                nc.scalar.activation(hc[:], zp[:], AF.Tanh)
                r = zr_sb[:, nt, :, HID:OC_G]
                t1 = epi_pool.tile([P, NB, HID], BF16, tag="t1", name=f"t1{nt}")
                nc.vector.tensor_sub(t1[:], stf[:, nt], hc[:])
                hf = epi_pool.tile([P, NB, HID], F32, tag="hf", name=f"hf{nt}")
                # h = hc + r*(state - hc)
                nc.vector.scalar_tensor_tensor(
                    out=hf[:], in0=t1[:], scalar=1.0, in1=r,
                    op0=ALU.mult, op1=ALU.mult,
                )
                nc.vector.tensor_add(hf[:], hf[:], hc[:])
                nc.sync.dma_start(
                    out_d[:, nt * P : (nt + 1) * P, :].rearrange("b p h -> p b h"),
                    hf[:],
                )

        # gconv 1 (gate), then gconv 2 (update); within each, the second
        # chain application is interleaved per-mt with that node tile's tail
        for gate in (True, False):
            apply1(first=gate)
            if gate:
                nc.vector.tensor_scalar_mul(rinv2[:], rinv[:], 2.0)
            for mt in range(NT):
                apply2(mt)
                tail_nt(mt, gate)


_NC = None


def _get_nc():
    global _NC
    if _NC is None:
        _NC = build_nc()
    return _NC


def _prep_in_maps(x, state, node_embeddings, W_gate, b_gate, W_update, b_update):
    bf = ml_dtypes.bfloat16
    x = np.asarray(x, dtype=np.float32)
    state = np.asarray(state, dtype=np.float32)
    E = np.asarray(node_embeddings, dtype=np.float32)
    W_gate = np.asarray(W_gate, dtype=np.float32)
    b_gate = np.asarray(b_gate, dtype=np.float32)
    W_update = np.asarray(W_update, dtype=np.float32)
    b_update = np.asarray(b_update, dtype=np.float32)

    eh = E.T.astype(bf)                       # [10, N] bf16
    el = (E.T - eh.astype(np.float32)).astype(bf)
    stack_l = np.concatenate([eh, el, eh], axis=0)   # lhsT rows
    stack_r = np.concatenate([eh, eh, el], axis=0)   # rhs rows
    et = np.ascontiguousarray(np.stack([stack_l, stack_r]))  # [2, 30, N]

    def wprep(W, b, oc):
        # W' rows: [bias | W(0:66) | W(66:132) | W(132:198) | zeros to 256]
        wp = np.zeros((256, oc), np.float32)
        wp[0] = b
        wp[1 : 1 + 3 * CH] = W
        return wp.reshape(2, 128, oc).astype(bf)

    wg = wprep(W_gate, b_gate, OC_G)
    wu = wprep(W_update, b_update, HID)
    ident = np.eye(P, dtype=bf)

    in_maps = []
    for r in range(NCORES):
        in_maps.append(
            {
                "x": np.ascontiguousarray(x[NB * r : NB * (r + 1)]),
                "state": np.ascontiguousarray(state[NB * r : NB * (r + 1)]),
                "et": et,
                "ident": ident,
                "wg": wg,
                "wu": wu,
            }
        )
    return in_maps


def run(trace=False, **inputs):
    nc = _get_nc()
    in_maps = _prep_in_maps(**inputs)
    res = run_bass_kernel_spmd(
        nc, in_maps, core_ids=list(range(NCORES)), trace=trace
    )
    out = np.concatenate([res.results[r]["out"] for r in range(NCORES)], axis=0)
    return out, res


def kernel(**inputs) -> np.ndarray:
    out, _ = run(trace=False, **inputs)
    return out

